# revision 23
# baseline (speedup 1.0000x reference)
"""Trainium2 Bass kernel for nn_Clustering (discriminative/lane clustering loss).

Strategy (8 NeuronCores, data parallel over batch, 2 images per core):
  Per image b the loss needs only 24 per-cluster statistics (c = 1..4):
    counts_c = sum_px [inst==c]                      (4)
    S_ce     = sum_px [inst==c] * binary * pred_e    (16)
    T_c      = sum_px [inst==c] * binary * |pred|^2  (4)

  Gram formulation: the 20 masked products S/T are inner products between
  mask planes q_c = [inst==c]*binary and value planes {pred_e, r=|pred|^2}
  over all pixels.  Feed the masks as the PE *stationary* (4 masks x 32
  w-offsets = 128 columns, reloaded per 32-column block) and stream the
  values as *moving* data [5 channels x 32 offsets = 160 columns]; the
  (wa==wb) diagonal of the accumulated [128,160] PSUM Gram holds the
  statistics.  The 16 mask*pred multiplies happen inside the systolic
  array, cutting DVE work ~3x vs elementwise product planes.

  Engine split per [128, 1024] tile:
    DVE : int->bf16 cast, v = inst*binary, 8 indicator compares (4 masked
          q_c on v, 4 raw ind_c on inst), 2 adds for r = sum_e pred_e^2
    ACT : binary + pred f32->bf16 casts, pred^2 squares, PSUM evacuation
    PE  : 32 Gram matmuls [128,(4,32)]^T @ [128,(5,32)] per tile, plus
          ones-column count reductions in 4 concurrent column groups
    DMA : 3 HWDGE loads per tile, 1 store per image
  The host reduces the Gram diagonal and evaluates the tiny [B,C,E] tail
  (means, variance hinge, pairwise center repulsion).
"""
import sys

sys.path.insert(0, '/opt/trn_rl_repo')

import numpy as np
from contextlib import ExitStack

import concourse.bass as bass
import concourse.mybir as mybir
import concourse.tile as tile
from concourse.alu_op_type import AluOpType
from concourse.vector_clock import ScopedClock

F32 = mybir.dt.float32
I32 = mybir.dt.int32
BF16 = mybir.dt.bfloat16

B, E, H, W = 16, 4, 512, 1024
NCORES = 8
B_LOC = B // NCORES          # images per core
C = 4                        # clusters 1..4 (background dropped)
HT = H // 128                # h-tiles per image
WB = 32                      # gram block width (one mask per 32-col PE group)
NB = W // WB                 # gram blocks per tile row
WH = W // 2                  # w-half width for pipeline drain
NV = 8                       # moving channels: pred_e x4, pred_e^2 x4
GW = NV * WB                 # gram psum cols = 256
OUTW = GW                    # out cols

DELTA_V = 0.5
DELTA_D = 3.0

# ---------------------------------------------------------------------------
# Toolchain workaround: this walrus build rejects instructions carrying more
# than one sem-wait ("Too many sync wait commands").  Keep 1 wait per
# instruction and spill the rest onto preceding same-engine NOPs (the engine
# executes them in order, so semantics are unchanged).
_MAX_WAITS = 1


def _split_waits_prepend(tc, inst):
    si = getattr(inst, 'sync_info', None)
    if si is None or not si.on_wait or len(si.on_wait) <= _MAX_WAITS:
        return
    if inst.engine == mybir.EngineType.Unassigned:
        return
    waits = list(si.on_wait)
    si.on_wait = waits[:_MAX_WAITS]
    inst.sync_info = si
    for i in range(_MAX_WAITS, len(waits), _MAX_WAITS):
        nop = mybir.InstNoOp(name=tc.nc.get_next_instruction_name(),
                             text_hint="wait_split")
        nop.engine = inst.engine
        nop.sync_info = mybir.SyncInfo(on_wait=waits[i:i + _MAX_WAITS],
                                       on_update=[])
        tc._add_instruction(nop)


_orig_commit_and_lower = tile.TileContext._commit_and_lower


def _patched_commit_and_lower(self, inst, original_block, old_bb_map,
                              bb_to_exit_bb):
    _split_waits_prepend(self, inst)
    return _orig_commit_and_lower(self, inst, original_block, old_bb_map,
                                  bb_to_exit_bb)


tile.TileContext._commit_and_lower = _patched_commit_and_lower


def _patched_drain_and_barrier(self, tick_clock, wait_clock):
    nc = self.nc
    drain_inst = nc.sync.drain()
    wait_clock.add_sem_waits(
        drain_inst.ins, ScopedClock({None: tick_clock.global_clock})
    )
    si = drain_inst.ins.sync_info
    if si is not None and si.on_wait and len(si.on_wait) > _MAX_WAITS:
        waits = list(si.on_wait)
        si.on_wait = waits[:_MAX_WAITS]
        drain_inst.ins.sync_info = si
        extra = waits[_MAX_WAITS:]
        for i in range(0, len(extra), _MAX_WAITS):
            nop = nc.sync.nop()
            nop.ins.sync_info = mybir.SyncInfo(
                on_wait=extra[i:i + _MAX_WAITS], on_update=[]
            )
    nc.all_engine_barrier()
    assert self.sems is not None
    popped = nc._tile_sem_poison_stack.pop()
    assert popped is self._sem_poison
    nc.clear_and_free_semaphores(list(self.sems.allocated().values()))
    nc.all_engine_barrier()


tile.TileContext._drain_and_barrier = _patched_drain_and_barrier
# ---------------------------------------------------------------------------


def _build_nc():
    nc = bass.Bass()
    pred = nc.dram_tensor("pred", [B_LOC, E, H, W], F32, kind="ExternalInput")
    binary = nc.dram_tensor("binary", [B_LOC, H, W], F32, kind="ExternalInput")
    inst = nc.dram_tensor("inst", [B_LOC, H, W], I32, kind="ExternalInput")
    out = nc.dram_tensor("out", [B_LOC, 128, OUTW], F32, kind="ExternalOutput")

    with tile.TileContext(nc) as tc:
        with ExitStack() as ctx:
            pred_pool = ctx.enter_context(tc.tile_pool(name="pred", bufs=3))
            in_pool = ctx.enter_context(tc.tile_pool(name="inp", bufs=3))
            bf_pool = ctx.enter_context(tc.tile_pool(name="bf", bufs=3))
            vals_pool = ctx.enter_context(tc.tile_pool(name="vals", bufs=3))
            mask_pool = ctx.enter_context(tc.tile_pool(name="mask", bufs=3))
            ps_pool = ctx.enter_context(
                tc.tile_pool(name="ps", bufs=2, space="PSUM"))
            out_pool = ctx.enter_context(tc.tile_pool(name="outp", bufs=2))

            for b in range(B_LOC):
                gram_ps = ps_pool.tile([128, GW], F32, tag="gram")
                for t in range(HT):
                    h0 = 128 * t
                    inst_t = in_pool.tile([128, W], I32, tag="inst")
                    nc.sync.dma_start(
                        out=inst_t[:], in_=inst[b, h0:h0 + 128, :])
                    bin_t = in_pool.tile([128, W], F32, tag="bin")
                    nc.sync.dma_start(
                        out=bin_t[:], in_=binary[b, h0:h0 + 128, :])
                    pred_t = pred_pool.tile([128, E, W], F32, tag="pred")
                    for e in range(E):
                        nc.sync.dma_start(
                            out=pred_t[:, e, :],
                            in_=pred[b, e, h0:h0 + 128, :])

                    # ACT: pred cast per e-pair; DVE: bin/inst casts + v.
                    vals = vals_pool.tile([128, NV, W], BF16, tag="vals")
                    nc.scalar.copy(vals[:, 0:2], pred_t[:, 0:2])
                    nc.scalar.copy(vals[:, 2:4], pred_t[:, 2:4])
                    bin_bf = bf_pool.tile([128, W], BF16, tag="binbf")
                    nc.vector.tensor_copy(bin_bf[:], bin_t[:])
                    inst_bf = bf_pool.tile([128, W], BF16, tag="instbf")
                    nc.vector.tensor_copy(inst_bf[:], inst_t[:])
                    v = bf_pool.tile([128, W], BF16, tag="v")
                    nc.vector.tensor_tensor(v[:], inst_bf[:], bin_bf[:],
                                            AluOpType.mult)
                    masks_q = mask_pool.tile([128, C, W], BF16, tag="mq")

                    # Squares, masks, and Gram matmuls run per w-half so the
                    # pipeline drains in half-tile latency at the end.
                    for wh in range(2):
                        sl = slice(WH * wh, WH * (wh + 1))
                        nc.scalar.activation(
                            vals[:, E + 2:E + 4, sl], vals[:, 2:4, sl],
                            mybir.ActivationFunctionType.Square)
                        nc.vector.tensor_tensor(
                            vals[:, E:E + 2, sl], vals[:, 0:2, sl],
                            vals[:, 0:2, sl], AluOpType.mult)
                        for c in range(C):
                            nc.vector.tensor_scalar(
                                masks_q[:, c, sl], v[:, sl], float(c + 1),
                                None, AluOpType.is_equal)
                        # PE: Gram blocks — masks^T @ vals, diagonal-extracted
                        # on the host.  Each mask goes to its own 32-wide PE
                        # column group so the 4 matmuls of a block stream
                        # concurrently through separate XBUSes and the
                        # [128,32] stationary loads shrink 4x.
                        for wb in range(NB // 2):
                            w0 = WH * wh + WB * wb
                            for c in range(C):
                                nc.tensor.matmul(
                                    gram_ps[32 * c:32 * c + WB, :],
                                    masks_q[:, c, w0:w0 + WB],
                                    vals[:, :, w0:w0 + WB],
                                    start=(t == 0 and wh == 0 and wb == 0),
                                    stop=(t == HT - 1 and wh == 1
                                          and wb == NB // 2 - 1),
                                    tile_position=(0, 32 * c),
                                )

                out_sb = out_pool.tile([128, OUTW], F32)
                nc.scalar.copy(out_sb[:], gram_ps[:])
                nc.sync.dma_start(out=out[b], in_=out_sb[:])
    return nc


_NC = None


def _get_nc():
    global _NC
    if _NC is None:
        _NC = _build_nc()
    return _NC


def _finalize(stats: np.ndarray, instance_label: np.ndarray) -> np.float32:
    """stats: [B, 128, OUTW] f32 gram, rows (c,wa) x cols (v,wb) -> loss.

    Moving channels v: 0..3 = pred_e, 4..7 = pred_e^2.  Cluster pixel
    counts are an O(N) int scan, done host-side."""
    s = stats.astype(np.float64)
    gram = s.reshape(B, C, WB, NV, WB)
    diag = np.einsum('bcwvw->bcv', gram)
    S = diag[:, :, 0:E]                           # [B, 4, 4]
    T = diag[:, :, E:].sum(-1)                    # [B, 4]
    flat = instance_label.reshape(B, -1)
    counts = np.stack(
        [np.bincount(flat[b], minlength=C + 1)[1:] for b in range(B)]
    ).astype(np.float64)                          # [B, 4]
    with np.errstate(divide='ignore', invalid='ignore'):
        mu = S / counts[..., None]
        ssd = np.maximum(T - counts * (mu * mu).sum(-1), 0.0)
        nrm = np.sqrt(ssd)
        var = np.where(nrm > DELTA_V, (nrm - DELTA_V) ** 2, 0.0)
        L_var = var.mean()
        diff = mu[:, :, None, :] - mu[:, None, :, :]
        d2 = (diff * diff).sum(-1)
        eye = np.eye(C, dtype=bool)
        dist = np.sqrt(np.where(eye, 1.0, d2))
        dloss = np.where(eye, 0.0,
                         np.maximum(DELTA_D - dist, 0.0) ** 2).sum((-1, -2))
        L_dist = dloss.mean()
    return np.float32(L_var + L_dist)


def kernel(pred: np.ndarray, binary_label: np.ndarray,
           instance_label: np.ndarray) -> np.ndarray:
    from concourse.bass_utils import run_bass_kernel_spmd

    nc = _get_nc()
    in_maps = []
    for core in range(NCORES):
        b0 = core * B_LOC
        in_maps.append({
            "pred": np.ascontiguousarray(pred[b0:b0 + B_LOC], dtype=np.float32),
            "binary": np.ascontiguousarray(
                binary_label[b0:b0 + B_LOC], dtype=np.float32),
            "inst": np.ascontiguousarray(
                instance_label[b0:b0 + B_LOC], dtype=np.int32),
        })
    res = run_bass_kernel_spmd(nc, in_maps, core_ids=list(range(NCORES)))
    stats = np.concatenate([res.results[c]["out"] for c in range(NCORES)],
                           axis=0)              # [B, 128, OUTW]
    return _finalize(stats, np.asarray(instance_label, dtype=np.int64))


# revision 24
# speedup vs baseline: 1.0355x; 1.0355x over previous
"""Trainium2 Bass kernel for nn_Clustering (discriminative/lane clustering loss).

Strategy (8 NeuronCores, data parallel over batch, 2 images per core):
  Per image b the loss needs only 24 per-cluster statistics (c = 1..4):
    counts_c = sum_px [inst==c]                      (4)
    S_ce     = sum_px [inst==c] * binary * pred_e    (16)
    T_c      = sum_px [inst==c] * binary * |pred|^2  (4)

  Gram formulation: the 20 masked products S/T are inner products between
  mask planes q_c = [inst==c]*binary and value planes {pred_e, r=|pred|^2}
  over all pixels.  Feed the masks as the PE *stationary* (4 masks x 32
  w-offsets = 128 columns, reloaded per 32-column block) and stream the
  values as *moving* data [5 channels x 32 offsets = 160 columns]; the
  (wa==wb) diagonal of the accumulated [128,160] PSUM Gram holds the
  statistics.  The 16 mask*pred multiplies happen inside the systolic
  array, cutting DVE work ~3x vs elementwise product planes.

  Engine split per [128, 1024] tile:
    DVE : int->bf16 cast, v = inst*binary, 8 indicator compares (4 masked
          q_c on v, 4 raw ind_c on inst), 2 adds for r = sum_e pred_e^2
    ACT : binary + pred f32->bf16 casts, pred^2 squares, PSUM evacuation
    PE  : 32 Gram matmuls [128,(4,32)]^T @ [128,(5,32)] per tile, plus
          ones-column count reductions in 4 concurrent column groups
    DMA : 3 HWDGE loads per tile, 1 store per image
  The host reduces the Gram diagonal and evaluates the tiny [B,C,E] tail
  (means, variance hinge, pairwise center repulsion).
"""
import sys

sys.path.insert(0, '/opt/trn_rl_repo')

import numpy as np
from contextlib import ExitStack

import concourse.bass as bass
import concourse.mybir as mybir
import concourse.tile as tile
from concourse.alu_op_type import AluOpType
from concourse.vector_clock import ScopedClock

F32 = mybir.dt.float32
I32 = mybir.dt.int32
BF16 = mybir.dt.bfloat16

B, E, H, W = 16, 4, 512, 1024
NCORES = 8
B_LOC = B // NCORES          # images per core
C = 4                        # clusters 1..4 (background dropped)
HT = H // 128                # h-tiles per image
WB = 32                      # gram block width (one mask per 32-col PE group)
NB = W // WB                 # gram blocks per tile row
WH = W // 2                  # w-half width for pipeline drain
NV = 8                       # moving channels: pred_e x4, pred_e^2 x4
GW = NV * WB                 # gram psum cols = 256
OUTW = GW                    # out cols

DELTA_V = 0.5
DELTA_D = 3.0

# ---------------------------------------------------------------------------
# Toolchain workaround: this walrus build rejects instructions carrying more
# than one sem-wait ("Too many sync wait commands").  Keep 1 wait per
# instruction and spill the rest onto preceding same-engine NOPs (the engine
# executes them in order, so semantics are unchanged).
_MAX_WAITS = 1


def _split_waits_prepend(tc, inst):
    si = getattr(inst, 'sync_info', None)
    if si is None or not si.on_wait or len(si.on_wait) <= _MAX_WAITS:
        return
    if inst.engine == mybir.EngineType.Unassigned:
        return
    waits = list(si.on_wait)
    si.on_wait = waits[:_MAX_WAITS]
    inst.sync_info = si
    for i in range(_MAX_WAITS, len(waits), _MAX_WAITS):
        nop = mybir.InstNoOp(name=tc.nc.get_next_instruction_name(),
                             text_hint="wait_split")
        nop.engine = inst.engine
        nop.sync_info = mybir.SyncInfo(on_wait=waits[i:i + _MAX_WAITS],
                                       on_update=[])
        tc._add_instruction(nop)


_orig_commit_and_lower = tile.TileContext._commit_and_lower


def _patched_commit_and_lower(self, inst, original_block, old_bb_map,
                              bb_to_exit_bb):
    _split_waits_prepend(self, inst)
    return _orig_commit_and_lower(self, inst, original_block, old_bb_map,
                                  bb_to_exit_bb)


tile.TileContext._commit_and_lower = _patched_commit_and_lower


def _patched_drain_and_barrier(self, tick_clock, wait_clock):
    nc = self.nc
    drain_inst = nc.sync.drain()
    wait_clock.add_sem_waits(
        drain_inst.ins, ScopedClock({None: tick_clock.global_clock})
    )
    si = drain_inst.ins.sync_info
    if si is not None and si.on_wait and len(si.on_wait) > _MAX_WAITS:
        waits = list(si.on_wait)
        si.on_wait = waits[:_MAX_WAITS]
        drain_inst.ins.sync_info = si
        extra = waits[_MAX_WAITS:]
        for i in range(0, len(extra), _MAX_WAITS):
            nop = nc.sync.nop()
            nop.ins.sync_info = mybir.SyncInfo(
                on_wait=extra[i:i + _MAX_WAITS], on_update=[]
            )
    nc.all_engine_barrier()
    assert self.sems is not None
    popped = nc._tile_sem_poison_stack.pop()
    assert popped is self._sem_poison
    nc.clear_and_free_semaphores(list(self.sems.allocated().values()))
    nc.all_engine_barrier()


tile.TileContext._drain_and_barrier = _patched_drain_and_barrier
# ---------------------------------------------------------------------------


def _build_nc():
    nc = bass.Bass()
    pred = nc.dram_tensor("pred", [B_LOC, E, H, W], F32, kind="ExternalInput")
    binary = nc.dram_tensor("binary", [B_LOC, H, W], F32, kind="ExternalInput")
    inst = nc.dram_tensor("inst", [B_LOC, H, W], I32, kind="ExternalInput")
    out = nc.dram_tensor("out", [B_LOC, 128, OUTW], F32, kind="ExternalOutput")

    with tile.TileContext(nc) as tc:
        with ExitStack() as ctx:
            pred_pool = ctx.enter_context(tc.tile_pool(name="pred", bufs=3))
            in_pool = ctx.enter_context(tc.tile_pool(name="inp", bufs=3))
            bf_pool = ctx.enter_context(tc.tile_pool(name="bf", bufs=3))
            vals_pool = ctx.enter_context(tc.tile_pool(name="vals", bufs=3))
            mask_pool = ctx.enter_context(tc.tile_pool(name="mask", bufs=3))
            ps_pool = ctx.enter_context(
                tc.tile_pool(name="ps", bufs=2, space="PSUM"))
            out_pool = ctx.enter_context(tc.tile_pool(name="outp", bufs=2))

            for b in range(B_LOC):
                gram_ps = ps_pool.tile([128, GW], F32, tag="gram")
                for t in range(HT):
                    h0 = 128 * t
                    inst_t = in_pool.tile([128, W], I32, tag="inst")
                    nc.sync.dma_start(
                        out=inst_t[:], in_=inst[b, h0:h0 + 128, :])
                    bin_t = in_pool.tile([128, W], F32, tag="bin")
                    nc.sync.dma_start(
                        out=bin_t[:], in_=binary[b, h0:h0 + 128, :])
                    pred_t = pred_pool.tile([128, E, W], F32, tag="pred")
                    nc.sync.dma_start(
                        out=pred_t[:],
                        in_=pred[b, :, h0:h0 + 128, :].rearrange(
                            "e h w -> h e w"),
                    )

                    # ACT: pred casts; DVE: bin/inst casts + v, squares, masks.
                    vals = vals_pool.tile([128, NV, W], BF16, tag="vals")
                    bin_bf = bf_pool.tile([128, W], BF16, tag="binbf")
                    nc.vector.tensor_copy(bin_bf[:], bin_t[:])
                    inst_bf = bf_pool.tile([128, W], BF16, tag="instbf")
                    nc.vector.tensor_copy(inst_bf[:], inst_t[:])
                    v = bf_pool.tile([128, W], BF16, tag="v")
                    nc.vector.tensor_tensor(v[:], inst_bf[:], bin_bf[:],
                                            AluOpType.mult)
                    masks_q = mask_pool.tile([128, C, W], BF16, tag="mq")

                    # Squares, masks, and Gram matmuls run per w-split so
                    # the pipeline drains in split-tile latency at the end;
                    # the final tile uses quarters for a short drain.
                    last = (b == B_LOC - 1 and t == HT - 1)
                    nsp = 4 if last else 2
                    wsp = W // nsp
                    for wh in range(nsp):
                        sl = slice(wsp * wh, wsp * (wh + 1))
                        nc.scalar.copy(vals[:, 0:E, sl], pred_t[:, :, sl])
                        nc.vector.tensor_tensor(
                            vals[:, E:, sl], vals[:, 0:E, sl],
                            vals[:, 0:E, sl], AluOpType.mult)
                        for c in range(C):
                            nc.vector.tensor_scalar(
                                masks_q[:, c, sl], v[:, sl], float(c + 1),
                                None, AluOpType.is_equal)
                        # PE: Gram blocks — masks^T @ vals, diagonal-extracted
                        # on the host.  Each mask goes to its own 32-wide PE
                        # column group so the 4 matmuls of a block stream
                        # concurrently through separate XBUSes and the
                        # [128,32] stationary loads shrink 4x.
                        for wb in range(NB // nsp):
                            w0 = wsp * wh + WB * wb
                            for c in range(C):
                                nc.tensor.matmul(
                                    gram_ps[32 * c:32 * c + WB, :],
                                    masks_q[:, c, w0:w0 + WB],
                                    vals[:, :, w0:w0 + WB],
                                    start=(t == 0 and wh == 0 and wb == 0),
                                    stop=(t == HT - 1 and wh == nsp - 1
                                          and wb == NB // nsp - 1),
                                    tile_position=(0, 32 * c),
                                )

                out_sb = out_pool.tile([128, OUTW], F32)
                nc.scalar.copy(out_sb[:], gram_ps[:])
                nc.sync.dma_start(out=out[b], in_=out_sb[:])
    return nc


_NC = None


def _get_nc():
    global _NC
    if _NC is None:
        _NC = _build_nc()
    return _NC


def _finalize(stats: np.ndarray, instance_label: np.ndarray) -> np.float32:
    """stats: [B, 128, OUTW] f32 gram, rows (c,wa) x cols (v,wb) -> loss.

    Moving channels v: 0..3 = pred_e, 4..7 = pred_e^2.  Cluster pixel
    counts are an O(N) int scan, done host-side."""
    s = stats.astype(np.float64)
    gram = s.reshape(B, C, WB, NV, WB)
    diag = np.einsum('bcwvw->bcv', gram)
    S = diag[:, :, 0:E]                           # [B, 4, 4]
    T = diag[:, :, E:].sum(-1)                    # [B, 4]
    flat = instance_label.reshape(B, -1)
    counts = np.stack(
        [np.bincount(flat[b], minlength=C + 1)[1:] for b in range(B)]
    ).astype(np.float64)                          # [B, 4]
    with np.errstate(divide='ignore', invalid='ignore'):
        mu = S / counts[..., None]
        ssd = np.maximum(T - counts * (mu * mu).sum(-1), 0.0)
        nrm = np.sqrt(ssd)
        var = np.where(nrm > DELTA_V, (nrm - DELTA_V) ** 2, 0.0)
        L_var = var.mean()
        diff = mu[:, :, None, :] - mu[:, None, :, :]
        d2 = (diff * diff).sum(-1)
        eye = np.eye(C, dtype=bool)
        dist = np.sqrt(np.where(eye, 1.0, d2))
        dloss = np.where(eye, 0.0,
                         np.maximum(DELTA_D - dist, 0.0) ** 2).sum((-1, -2))
        L_dist = dloss.mean()
    return np.float32(L_var + L_dist)


def kernel(pred: np.ndarray, binary_label: np.ndarray,
           instance_label: np.ndarray) -> np.ndarray:
    from concourse.bass_utils import run_bass_kernel_spmd

    nc = _get_nc()
    in_maps = []
    for core in range(NCORES):
        b0 = core * B_LOC
        in_maps.append({
            "pred": np.ascontiguousarray(pred[b0:b0 + B_LOC], dtype=np.float32),
            "binary": np.ascontiguousarray(
                binary_label[b0:b0 + B_LOC], dtype=np.float32),
            "inst": np.ascontiguousarray(
                instance_label[b0:b0 + B_LOC], dtype=np.int32),
        })
    res = run_bass_kernel_spmd(nc, in_maps, core_ids=list(range(NCORES)))
    stats = np.concatenate([res.results[c]["out"] for c in range(NCORES)],
                           axis=0)              # [B, 128, OUTW]
    return _finalize(stats, np.asarray(instance_label, dtype=np.int64))


# revision 25
# speedup vs baseline: 1.1001x; 1.0624x over previous
"""Trainium2 Bass kernel for nn_Clustering (discriminative/lane clustering loss).

Strategy (8 NeuronCores, data parallel over batch, 2 images per core):
  Per image b the loss needs only 24 per-cluster statistics (c = 1..4):
    counts_c = sum_px [inst==c]                      (4)
    S_ce     = sum_px [inst==c] * binary * pred_e    (16)
    T_c      = sum_px [inst==c] * binary * |pred|^2  (4)

  Gram formulation: the 20 masked products S/T are inner products between
  mask planes q_c = [inst==c]*binary and value planes {pred_e, r=|pred|^2}
  over all pixels.  Feed the masks as the PE *stationary* (4 masks x 32
  w-offsets = 128 columns, reloaded per 32-column block) and stream the
  values as *moving* data [5 channels x 32 offsets = 160 columns]; the
  (wa==wb) diagonal of the accumulated [128,160] PSUM Gram holds the
  statistics.  The 16 mask*pred multiplies happen inside the systolic
  array, cutting DVE work ~3x vs elementwise product planes.

  Engine split per [128, 1024] tile:
    DVE : int->bf16 cast, v = inst*binary, 8 indicator compares (4 masked
          q_c on v, 4 raw ind_c on inst), 2 adds for r = sum_e pred_e^2
    ACT : binary + pred f32->bf16 casts, pred^2 squares, PSUM evacuation
    PE  : 32 Gram matmuls [128,(4,32)]^T @ [128,(5,32)] per tile, plus
          ones-column count reductions in 4 concurrent column groups
    DMA : 3 HWDGE loads per tile, 1 store per image
  The host reduces the Gram diagonal and evaluates the tiny [B,C,E] tail
  (means, variance hinge, pairwise center repulsion).
"""
import sys

sys.path.insert(0, '/opt/trn_rl_repo')

import numpy as np
from contextlib import ExitStack

import concourse.bass as bass
import concourse.mybir as mybir
import concourse.tile as tile
from concourse.alu_op_type import AluOpType
from concourse.vector_clock import ScopedClock

F32 = mybir.dt.float32
I32 = mybir.dt.int32
BF16 = mybir.dt.bfloat16

B, E, H, W = 16, 4, 512, 1024
NCORES = 8
B_LOC = B // NCORES          # images per core
C = 4                        # clusters 1..4 (background dropped)
HT = H // 128                # h-tiles per image
WB = 32                      # gram block width (one mask per 32-col PE group)
NB = W // WB                 # gram blocks per tile row
WH = W // 2                  # w-half width for pipeline drain
NV = 8                       # moving channels: pred_e x4, pred_e^2 x4
GW = NV * WB                 # gram psum cols = 256
OUTW = GW                    # out cols

DELTA_V = 0.5
DELTA_D = 3.0

# ---------------------------------------------------------------------------
# Toolchain workaround: this walrus build rejects instructions carrying more
# than one sem-wait ("Too many sync wait commands").  Keep 1 wait per
# instruction and spill the rest onto preceding same-engine NOPs (the engine
# executes them in order, so semantics are unchanged).
_MAX_WAITS = 1


def _split_waits_prepend(tc, inst):
    si = getattr(inst, 'sync_info', None)
    if si is None or not si.on_wait or len(si.on_wait) <= _MAX_WAITS:
        return
    if inst.engine == mybir.EngineType.Unassigned:
        return
    waits = list(si.on_wait)
    si.on_wait = waits[:_MAX_WAITS]
    inst.sync_info = si
    for i in range(_MAX_WAITS, len(waits), _MAX_WAITS):
        nop = mybir.InstNoOp(name=tc.nc.get_next_instruction_name(),
                             text_hint="wait_split")
        nop.engine = inst.engine
        nop.sync_info = mybir.SyncInfo(on_wait=waits[i:i + _MAX_WAITS],
                                       on_update=[])
        tc._add_instruction(nop)


_orig_commit_and_lower = tile.TileContext._commit_and_lower


def _patched_commit_and_lower(self, inst, original_block, old_bb_map,
                              bb_to_exit_bb):
    _split_waits_prepend(self, inst)
    return _orig_commit_and_lower(self, inst, original_block, old_bb_map,
                                  bb_to_exit_bb)


tile.TileContext._commit_and_lower = _patched_commit_and_lower


def _patched_drain_and_barrier(self, tick_clock, wait_clock):
    nc = self.nc
    drain_inst = nc.sync.drain()
    wait_clock.add_sem_waits(
        drain_inst.ins, ScopedClock({None: tick_clock.global_clock})
    )
    si = drain_inst.ins.sync_info
    if si is not None and si.on_wait and len(si.on_wait) > _MAX_WAITS:
        waits = list(si.on_wait)
        si.on_wait = waits[:_MAX_WAITS]
        drain_inst.ins.sync_info = si
        extra = waits[_MAX_WAITS:]
        for i in range(0, len(extra), _MAX_WAITS):
            nop = nc.sync.nop()
            nop.ins.sync_info = mybir.SyncInfo(
                on_wait=extra[i:i + _MAX_WAITS], on_update=[]
            )
    nc.all_engine_barrier()
    assert self.sems is not None
    popped = nc._tile_sem_poison_stack.pop()
    assert popped is self._sem_poison
    nc.clear_and_free_semaphores(list(self.sems.allocated().values()))
    nc.all_engine_barrier()


tile.TileContext._drain_and_barrier = _patched_drain_and_barrier
# ---------------------------------------------------------------------------


def _build_nc():
    nc = bass.Bass()
    pred = nc.dram_tensor("pred", [B_LOC, E, H, W], F32, kind="ExternalInput")
    binary = nc.dram_tensor("binary", [B_LOC, H, W], F32, kind="ExternalInput")
    inst = nc.dram_tensor("inst", [B_LOC, H, W], I32, kind="ExternalInput")
    out = nc.dram_tensor("out", [B_LOC, 128, OUTW], F32, kind="ExternalOutput")

    with tile.TileContext(nc) as tc:
        with ExitStack() as ctx:
            pred_pool = ctx.enter_context(tc.tile_pool(name="pred", bufs=3))
            in_pool = ctx.enter_context(tc.tile_pool(name="inp", bufs=3))
            bf_pool = ctx.enter_context(tc.tile_pool(name="bf", bufs=3))
            vals_pool = ctx.enter_context(tc.tile_pool(name="vals", bufs=3))
            mask_pool = ctx.enter_context(tc.tile_pool(name="mask", bufs=3))
            ps_pool = ctx.enter_context(
                tc.tile_pool(name="ps", bufs=2, space="PSUM"))
            out_pool = ctx.enter_context(tc.tile_pool(name="outp", bufs=2))

            for b in range(B_LOC):
                gram_ps = ps_pool.tile([128, GW], F32, tag="gram")
                for t in range(HT):
                    h0 = 128 * t
                    inst_t = in_pool.tile([128, W], I32, tag="inst")
                    nc.sync.dma_start(
                        out=inst_t[:], in_=inst[b, h0:h0 + 128, :])
                    bin_t = in_pool.tile([128, W], F32, tag="bin")
                    nc.sync.dma_start(
                        out=bin_t[:], in_=binary[b, h0:h0 + 128, :])
                    pred_t = pred_pool.tile([128, E, W], F32, tag="pred")
                    nc.sync.dma_start(
                        out=pred_t[:],
                        in_=pred[b, :, h0:h0 + 128, :].rearrange(
                            "e h w -> h e w"),
                    )

                    # ACT: pred casts; DVE: bin/inst casts + v, squares, masks.
                    vals = vals_pool.tile([128, NV, W], BF16, tag="vals")
                    bin_bf = bf_pool.tile([128, W], BF16, tag="binbf")
                    nc.vector.tensor_copy(bin_bf[:], bin_t[:])
                    inst_bf = bf_pool.tile([128, W], BF16, tag="instbf")
                    nc.vector.tensor_copy(inst_bf[:], inst_t[:])
                    v = bf_pool.tile([128, W], BF16, tag="v")
                    nc.vector.tensor_tensor(v[:], inst_bf[:], bin_bf[:],
                                            AluOpType.mult)
                    masks_q = mask_pool.tile([128, C, W], BF16, tag="mq")

                    # Squares, masks, and Gram matmuls run per w-split so
                    # the pipeline drains in split-tile latency at the end;
                    # the final tile uses quarters for a short drain.
                    last = (b == B_LOC - 1 and t == HT - 1)
                    nsp = 4 if last else 2
                    wsp = W // nsp
                    for wh in range(nsp):
                        sl = slice(wsp * wh, wsp * (wh + 1))
                        nc.scalar.copy(vals[:, 0:E, sl], pred_t[:, :, sl])
                        nc.scalar.activation(
                            vals[:, E + 2:E + 4, sl], vals[:, 2:4, sl],
                            mybir.ActivationFunctionType.Square)
                        nc.vector.tensor_tensor(
                            vals[:, E:E + 2, sl], vals[:, 0:2, sl],
                            vals[:, 0:2, sl], AluOpType.mult)
                        for c in range(C):
                            nc.vector.tensor_scalar(
                                masks_q[:, c, sl], v[:, sl], float(c + 1),
                                None, AluOpType.is_equal)
                        # PE: Gram blocks — masks^T @ vals, diagonal-extracted
                        # on the host.  Each mask goes to its own 32-wide PE
                        # column group so the 4 matmuls of a block stream
                        # concurrently through separate XBUSes and the
                        # [128,32] stationary loads shrink 4x.
                        for wb in range(NB // nsp):
                            w0 = wsp * wh + WB * wb
                            for c in range(C):
                                nc.tensor.matmul(
                                    gram_ps[32 * c:32 * c + WB, :],
                                    masks_q[:, c, w0:w0 + WB],
                                    vals[:, :, w0:w0 + WB],
                                    start=(t == 0 and wh == 0 and wb == 0),
                                    stop=(t == HT - 1 and wh == nsp - 1
                                          and wb == NB // nsp - 1),
                                    tile_position=(0, 32 * c),
                                )

                out_sb = out_pool.tile([128, OUTW], F32)
                nc.scalar.copy(out_sb[:], gram_ps[:])
                nc.sync.dma_start(out=out[b], in_=out_sb[:])
    return nc


_NC = None


def _get_nc():
    global _NC
    if _NC is None:
        _NC = _build_nc()
    return _NC


def _finalize(stats: np.ndarray, instance_label: np.ndarray) -> np.float32:
    """stats: [B, 128, OUTW] f32 gram, rows (c,wa) x cols (v,wb) -> loss.

    Moving channels v: 0..3 = pred_e, 4..7 = pred_e^2.  Cluster pixel
    counts are an O(N) int scan, done host-side."""
    s = stats.astype(np.float64)
    gram = s.reshape(B, C, WB, NV, WB)
    diag = np.einsum('bcwvw->bcv', gram)
    S = diag[:, :, 0:E]                           # [B, 4, 4]
    T = diag[:, :, E:].sum(-1)                    # [B, 4]
    flat = instance_label.reshape(B, -1)
    counts = np.stack(
        [np.bincount(flat[b], minlength=C + 1)[1:] for b in range(B)]
    ).astype(np.float64)                          # [B, 4]
    with np.errstate(divide='ignore', invalid='ignore'):
        mu = S / counts[..., None]
        ssd = np.maximum(T - counts * (mu * mu).sum(-1), 0.0)
        nrm = np.sqrt(ssd)
        var = np.where(nrm > DELTA_V, (nrm - DELTA_V) ** 2, 0.0)
        L_var = var.mean()
        diff = mu[:, :, None, :] - mu[:, None, :, :]
        d2 = (diff * diff).sum(-1)
        eye = np.eye(C, dtype=bool)
        dist = np.sqrt(np.where(eye, 1.0, d2))
        dloss = np.where(eye, 0.0,
                         np.maximum(DELTA_D - dist, 0.0) ** 2).sum((-1, -2))
        L_dist = dloss.mean()
    return np.float32(L_var + L_dist)


def kernel(pred: np.ndarray, binary_label: np.ndarray,
           instance_label: np.ndarray) -> np.ndarray:
    from concourse.bass_utils import run_bass_kernel_spmd

    nc = _get_nc()
    in_maps = []
    for core in range(NCORES):
        b0 = core * B_LOC
        in_maps.append({
            "pred": np.ascontiguousarray(pred[b0:b0 + B_LOC], dtype=np.float32),
            "binary": np.ascontiguousarray(
                binary_label[b0:b0 + B_LOC], dtype=np.float32),
            "inst": np.ascontiguousarray(
                instance_label[b0:b0 + B_LOC], dtype=np.int32),
        })
    res = run_bass_kernel_spmd(nc, in_maps, core_ids=list(range(NCORES)))
    stats = np.concatenate([res.results[c]["out"] for c in range(NCORES)],
                           axis=0)              # [B, 128, OUTW]
    return _finalize(stats, np.asarray(instance_label, dtype=np.int64))
